# revision 8
# baseline (speedup 1.0000x reference)
"""GRU kernel for Trainium2, 8 NeuronCores, data-parallel over batch.

Problem: B=256, T=512, INPUT=128, HIDDEN=256, PyTorch gate order (r, z, n):
    r = sigmoid(W_ir x + b_ir + W_hr h + b_hr)
    z = sigmoid(W_iz x + b_iz + W_hz h + b_hz)
    n = tanh(W_in x + b_in + r * (W_hn h + b_hn))
    h' = (1 - z) n + z h
Outputs all hidden states [B, T, H].

Design (per core, B_loc=32 split into 2 independent streams of 16):
- "Transposed/wide" layout: SBUF tiles [128 partitions = hidden-dim half,
  free = 2 halves x 16 batch].  Gate elementwise ops are [128, 32] tiles.
- Input projections xg = W_ih x (+ biases) computed as a bulk GEMM per
  T-chunk (Tc=32), written into per-step layout buffers via ScalarE
  Identity-with-bias copies from PSUM.
- Per step: PSUM bank per stream is preloaded with xg' (r,z slots) and
  b_hn broadcast (n slot) via identity matmuls (TensorE writes PSUM with
  start=True), then 12 W_hh matmuls accumulate on top (start=False).
  Gates: fused sigmoid over r|z slots (ScalarE, from PSUM), n-chain and
  h' on VectorE:  m = r * psum_n;  pre_n = m + xgn';  n = tanh(pre_n);
  h' = n + z * (h_prev - n).
- h' written straight into the out-chunk buffer (doubles as h state),
  DMA'd to DRAM per chunk.
"""

import sys
import os
import numpy as np

for _p in ("/opt/trn_rl_repo", "/root/.axon_site/_ro/trn_rl_repo"):
    if os.path.isdir(_p) and _p not in sys.path:
        sys.path.insert(0, _p)

from concourse import bass, bacc, tile, mybir  # noqa: E402
from concourse.bass_utils import run_bass_kernel_spmd  # noqa: E402

B, T_FULL, IN, H = 256, 512, 128, 256
N_CORES = 8
B_LOC = B // N_CORES          # 32
NS = 2                        # batch streams per core
BS = B_LOC // NS              # 16
TC = 32                       # time-chunk length
F32 = mybir.dt.float32
BF16 = mybir.dt.bfloat16

# dtype knobs
MM_DT = F32      # matmul operand dtype (weights, h, xg-preload, x)
H_DT = F32       # h state / output-buffer dtype
GATE_DT = F32    # gate intermediate dtype

AF = mybir.ActivationFunctionType


def _np_dt(dt):
    return np.float32 if dt == F32 else np.dtype("bfloat16") if hasattr(np, "bfloat16") else np.float32


def build(t_len=T_FULL):
    """Build the Bass module for a per-core GRU over t_len steps."""
    assert t_len % TC == 0
    nchunk = t_len // TC
    nc = bacc.Bacc("TRN2", target_bir_lowering=False, debug=False,
                   num_devices=N_CORES)

    xt = nc.dram_tensor("xt", [IN, t_len, B_LOC], MM_DT, kind="ExternalInput")
    wih_t = nc.dram_tensor("wih_t", [3, 2, IN, 128], MM_DT, kind="ExternalInput")
    whh_t = nc.dram_tensor("whh_t", [3, 2, 2, 128, 128], MM_DT, kind="ExternalInput")
    bias_x = nc.dram_tensor("bias_x", [3, 2, 128, 1], F32, kind="ExternalInput")
    bhn_w = nc.dram_tensor("bhn_w", [128, 2 * BS], MM_DT, kind="ExternalInput")
    ident_d = nc.dram_tensor("ident", [128, 128], MM_DT, kind="ExternalInput")
    # [stream, hidden-half, hidden-within-half, t, batch] — partition-major
    # so the chunk store DMA balances to [p][t][b-contig].
    out_loc = nc.dram_tensor("out_loc", [NS, 2, 128, t_len, BS], H_DT,
                             kind="ExternalOutput")

    W = 2 * BS  # wide free size (32)

    from contextlib import ExitStack
    with tile.TileContext(nc) as tc, ExitStack() as es:
        cpool = es.enter_context(tc.tile_pool(name="consts", bufs=1))
        xpool = es.enter_context(tc.tile_pool(name="xp", bufs=2))
        rzpool = es.enter_context(tc.tile_pool(name="rzp", bufs=2))
        xgnpool = es.enter_context(tc.tile_pool(name="xgnp", bufs=2))
        outpool = es.enter_context(tc.tile_pool(name="outp", bufs=2))
        gpool = es.enter_context(tc.tile_pool(name="gp", bufs=3))
        psb = es.enter_context(tc.tile_pool(name="psb", bufs=2, space="PSUM"))
        pss = es.enter_context(tc.tile_pool(name="pss", bufs=3, space="PSUM"))

        # ---- constants into SBUF ----
        whh_sb = cpool.tile([128, 12 * 128], MM_DT)
        for g in range(3):
            for mh in range(2):
                for kc in range(2):
                    idx = (g * 2 + mh) * 2 + kc
                    nc.gpsimd.dma_start(whh_sb[:, idx * 128:(idx + 1) * 128],
                                        whh_t[g, mh, kc])
        wih_sb = cpool.tile([128, 6 * 128], MM_DT)
        for g in range(3):
            for mh in range(2):
                idx = g * 2 + mh
                nc.gpsimd.dma_start(wih_sb[:, idx * 128:(idx + 1) * 128],
                                    wih_t[g, mh])
        ident = cpool.tile([128, 128], MM_DT)
        nc.gpsimd.dma_start(ident[:], ident_d[:])
        bhn_sb = cpool.tile([128, W], MM_DT)
        nc.gpsimd.dma_start(bhn_sb[:], bhn_w[:])
        biasx_sb = cpool.tile([128, 6], F32)
        for g in range(3):
            for mh in range(2):
                idx = g * 2 + mh
                nc.gpsimd.dma_start(biasx_sb[:, idx:idx + 1], bias_x[g, mh])
        h0 = cpool.tile([128, W], H_DT)
        nc.vector.memset(h0[:], 0.0)

        h_prev = [h0, h0]
        h_prev_sl = [h0[:], h0[:]]

        for c in range(nchunk):
            t0 = c * TC
            rz_t = []
            xgn_t = []
            out_b = []
            for s in range(NS):
                x_t = xpool.tile([IN, TC, BS], MM_DT, tag=f"x{s}")
                nc.gpsimd.dma_start(
                    x_t[:], xt[:, t0:t0 + TC, s * BS:(s + 1) * BS])
                rz = rzpool.tile([128, TC, 2 * W], MM_DT, tag=f"rz{s}")
                xgn = xgnpool.tile([128, TC, W], F32, tag=f"xgn{s}")
                ob = outpool.tile([128, TC, W], H_DT, tag=f"ob{s}")
                rz_t.append(rz)
                xgn_t.append(xgn)
                out_b.append(ob)
                # bulk input-projection GEMM for this chunk+stream
                for g in range(3):
                    for mh in range(2):
                        idx = g * 2 + mh
                        ps = psb.tile([128, TC * BS], F32, tag="psb")
                        nc.tensor.matmul(
                            ps[:],
                            wih_sb[:, idx * 128:(idx + 1) * 128],
                            x_t[:],
                            start=True, stop=True)
                        if g < 2:
                            dst = rz[:, :, g * W + mh * BS: g * W + mh * BS + BS]
                        else:
                            dst = xgn[:, :, mh * BS:(mh + 1) * BS]
                        nc.scalar.activation(
                            dst,
                            ps[:].rearrange("p (t j) -> p t j", t=TC),
                            AF.Identity,
                            bias=biasx_sb[:, idx:idx + 1])

            for ti in range(TC):
                t = t0 + ti
                for s in range(NS):
                    ps = pss.tile([128, 3 * W], F32, tag=f"ps{s}")
                    # PSUM preload: xg' for r,z slots; b_hn bcast for n slot
                    nc.tensor.matmul(ps[:, 0:2 * W], ident[:],
                                     rz_t[s][:, ti, :], start=True, stop=False)
                    # start=False: bank bits were cleared by the first
                    # preload's start=True, so this overwrites-and-sets.
                    nc.tensor.matmul(ps[:, 2 * W:3 * W], ident[:],
                                     bhn_sb[:], start=False, stop=False)
                    # recurrent matmuls: accumulate W_hh @ h
                    for g in range(3):
                        for mh in range(2):
                            for kc in range(2):
                                idx = (g * 2 + mh) * 2 + kc
                                nc.tensor.matmul(
                                    ps[:, g * W + mh * BS:
                                       g * W + mh * BS + BS],
                                    whh_sb[:, idx * 128:(idx + 1) * 128],
                                    h_prev_sl[s][:, kc * BS:(kc + 1) * BS],
                                    start=False, stop=(kc == 1))
                    # gates
                    rz_sb = gpool.tile([128, 2 * W], GATE_DT, tag=f"g{s}")
                    nc.scalar.activation(rz_sb[:], ps[:, 0:2 * W], AF.Sigmoid)
                    m_sb = gpool.tile([128, W], F32, tag=f"m{s}")
                    nc.vector.tensor_mul(m_sb[:], ps[:, 2 * W:3 * W],
                                         rz_sb[:, 0:W])
                    pren = gpool.tile([128, W], F32, tag=f"pn{s}")
                    nc.vector.tensor_add(pren[:], m_sb[:], xgn_t[s][:, ti, :])
                    n_sb = gpool.tile([128, W], GATE_DT, tag=f"n{s}")
                    nc.scalar.activation(n_sb[:], pren[:], AF.Tanh)
                    d_sb = gpool.tile([128, W], GATE_DT, tag=f"d{s}")
                    nc.vector.tensor_sub(d_sb[:], h_prev_sl[s], n_sb[:])
                    e_sb = gpool.tile([128, W], GATE_DT, tag=f"e{s}")
                    nc.vector.tensor_mul(e_sb[:], rz_sb[:, W:2 * W], d_sb[:])
                    nc.vector.tensor_add(out_b[s][:, ti, :], n_sb[:], e_sb[:])
                    h_prev[s] = out_b[s]
                    h_prev_sl[s] = out_b[s][:, ti, :]

            # store chunk: out_b[s] [128, TC, 2*BS] -> out_loc[s, b, t, h]
            for s in range(NS):
                for hh in range(2):
                    dst = out_loc[s, hh, :, t0:t0 + TC, :]
                    src = out_b[s][:, :, hh * BS:(hh + 1) * BS]
                    nc.gpsimd.dma_start(dst, src)

    nc.compile()
    return nc


def _prep_core_inputs(x_c, W_ih, W_hh, b_ih, b_hh):
    """Host-side reshapes for one core's batch shard x_c [B_LOC, T, IN]."""
    t_len = x_c.shape[1]
    np_mm = np.float32
    xt = np.ascontiguousarray(x_c.transpose(2, 1, 0)).astype(np_mm)
    wih_t = np.ascontiguousarray(
        W_ih.reshape(3, 2, 128, IN).transpose(0, 1, 3, 2)).astype(np_mm)
    whh_t = np.ascontiguousarray(
        W_hh.reshape(3, 2, 128, 2, 128).transpose(0, 1, 3, 4, 2)).astype(np_mm)
    bsum = (b_ih + b_hh).astype(np.float32)
    bias_x = np.empty((3, 2, 128, 1), np.float32)
    for g in range(3):
        for mh in range(2):
            lo = g * 256 + mh * 128
            src = bsum if g < 2 else b_ih
            bias_x[g, mh, :, 0] = src[lo:lo + 128]
    bh = b_hh[512:768].reshape(2, 128)
    bhn_w = np.empty((128, 2 * BS), np.float32)
    bhn_w[:, :BS] = bh[0][:, None]
    bhn_w[:, BS:] = bh[1][:, None]
    ident = np.eye(128, dtype=np_mm)
    return {"xt": xt, "wih_t": wih_t, "whh_t": whh_t, "bias_x": bias_x,
            "bhn_w": bhn_w.astype(np_mm), "ident": ident}


_NC_CACHE = {}


def _get_nc(t_len):
    if t_len not in _NC_CACHE:
        _NC_CACHE[t_len] = build(t_len)
    return _NC_CACHE[t_len]


def kernel(x, W_ih, W_hh, b_ih, b_hh):
    x = np.asarray(x, np.float32)
    W_ih = np.asarray(W_ih, np.float32)
    W_hh = np.asarray(W_hh, np.float32)
    b_ih = np.asarray(b_ih, np.float32)
    b_hh = np.asarray(b_hh, np.float32)
    t_len = x.shape[1]
    nc = _get_nc(t_len)
    in_maps = []
    for c in range(N_CORES):
        x_c = x[c * B_LOC:(c + 1) * B_LOC]
        in_maps.append(_prep_core_inputs(x_c, W_ih, W_hh, b_ih, b_hh))
    res = run_bass_kernel_spmd(nc, in_maps, list(range(N_CORES)))
    out = np.empty((x.shape[0], t_len, H), np.float32)
    for c in range(N_CORES):
        ol = np.asarray(res.results[c]["out_loc"], np.float32)
        # [s, hh, p, t, b] -> [s, b, t, hh*128+p]
        ol = ol.transpose(0, 4, 3, 1, 2).reshape(NS, BS, t_len, H)
        for s in range(NS):
            out[c * B_LOC + s * BS: c * B_LOC + (s + 1) * BS] = ol[s]
    return out


def _np_gru(x, W_ih, W_hh, b_ih, b_hh):
    Bsz, t_len, _ = x.shape
    h = np.zeros((Bsz, H), np.float32)
    xg = x @ W_ih.T + b_ih
    out = np.empty((Bsz, t_len, H), np.float32)
    sig = lambda v: 1.0 / (1.0 + np.exp(-v))
    for t in range(t_len):
        hg = h @ W_hh.T + b_hh
        xr, xz, xn = np.split(xg[:, t], 3, -1)
        hr, hz, hn = np.split(hg, 3, -1)
        r = sig(xr + hr)
        z = sig(xz + hz)
        n = np.tanh(xn + r * hn)
        h = (1 - z) * n + z * h
        out[:, t] = h
    return out


if __name__ == "__main__":
    t_len = int(sys.argv[1]) if len(sys.argv) > 1 else 64
    rng = np.random.default_rng(0)
    s = 1.0 / np.sqrt(H)
    x = rng.standard_normal((B, t_len, IN), dtype=np.float32)
    W_ih = (rng.standard_normal((3 * H, IN)) * s).astype(np.float32)
    W_hh = (rng.standard_normal((3 * H, H)) * s).astype(np.float32)
    b_ih = (rng.standard_normal(3 * H) * s).astype(np.float32)
    b_hh = (rng.standard_normal(3 * H) * s).astype(np.float32)
    got = kernel(x, W_ih, W_hh, b_ih, b_hh)
    want = _np_gru(x, W_ih, W_hh, b_ih, b_hh)
    err = np.max(np.abs(got - want)) / max(1e-9, np.max(np.abs(want)))
    print("max:", np.max(np.abs(want)), "absmax diff:",
          np.max(np.abs(got - want)), "rel:", err)
    assert err < 2e-2, "FAIL"
    print("PASS")
